# revision 9
# baseline (speedup 1.0000x reference)
"""KAN expert kernel for Trainium2 (8 NeuronCores, data-parallel over batch).

Math: out[b,j] = sum_{i,g} basis_g(x[b,i]) * coeff[i,j,g] * scaling[i,j]
with cubic B-spline basis on the uniform extended grid g_m = -1 + 0.4*m.

Key identity (truncated powers): for the uniform grid, the basis is the
cardinal cubic B-spline, basis_g(x) = (1/(6h^3)) * sum_{r=0..4} w_r *
relu(x - g_{g+r})^3 with w = [1,-4,6,-4,1]. Since x in [-1,1) only
relu-features m=0..4 are nonzero, and the (linear) binomial combine is
folded into the weights on the host:
    C'[m,i,j] = (1/(6h^3)) * sum_g w_{m-g} * coeff[i,j,g] * scaling[i,j]
so each core computes Q_m = relu(x - g_m)^3 (m=0..4) and a
[512b x 2560k] @ [2560k x 512j] fp16 matmul accumulated in fp32 PSUM.

Precision: the truncated-power split cancels heavily, so the matmul
INPUTS need >= 10 mantissa bits: fp16 passes (7.8e-3 rel vs the 2e-2
gate) IFF the features are computed in fp32 and rounded to fp16 once:
    r_m = max(x - g_m, 0)     (DVE tensor_scalar, fp32)
    s_m = Square(r_m)         (ACT, fp32; == (x-g)^2 wherever r>0,
                               and q=0 elsewhere anyway -> exact)
    q_m = fp16(r_m * s_m)     (DVE tensor_mul, single rounding)
Using Square(r) instead of Square(x - g) needs NO bias constants, so
the kernel has no pre-TileContext memsets/barrier: the measured window
(first useful instruction -> teardown end) starts at Bass's builtin
const memsets and the X DMA issues ~0.9us earlier than with the
const+barrier prologue.

Schedule (measured on HW):
 - X lands in two pieces: a small ic0 piece (sync queue) that gates the
   first feature chunk, and the rest (scalar queue). W groups chain
   behind them on two queues so early tensors get full DMA bandwidth.
 - first feature chunks are 512 wide to minimize the X->first-MM
   latency; later chunks 1024/2048 (cheaper per element).
 - LDWEIGHTS+MATMUL pairs sustain ~216ns/MM when fed; each half carries
   at most one sync wait (q-producer on LDWEIGHTS, W-arrival DMA on
   MATMUL). PE declocks 2x if it idles >3.4us -> dummy warmup matmuls
   run while the DMAs land.
 - a generic wait-domination pass strips every sync wait already
   covered by an earlier same-engine wait (engines are in-order FIFOs),
   leaving <=1 sync wait per instruction for walrus.
"""

import numpy as np

BATCH = 4096
IN_DIM = 512
OUT_DIM = 512
GRID_SIZE = 5
K = 3
N_CORES = 8
P = 128
NM = 5                      # relu^3 feature channels
BC = BATCH // N_CORES       # 512 batch rows per core
NIC = IN_DIM // P           # 4 input-dim chunks
NBC = BC // P               # 4 batch chunks (psum tiles)
W_TOT = NIC * BC            # 2048 feature columns per tile

_W_BINOM = np.array([1.0, -4.0, 6.0, -4.0, 1.0])

_cached = {}


def _grid_f32():
    h = 2.0 / GRID_SIZE
    return np.float32(-1.0 + h * np.arange(GRID_SIZE + 2 * K + 1))


# per-m column chunking of the [128, 2048] feature space (ic-major, so
# [0:512] is exactly the ic0 block the first matmuls need)
_CHUNKS = {
    0: [(0, 512), (512, 1024), (1024, 2048)],
    1: [(0, 1024), (1024, 2048)],
    2: [(0, 1024), (1024, 2048)],
    3: [(0, 1024), (1024, 2048)],
    4: [(0, 1024), (1024, 2048)],
}
# Q (3-stream tensor_tensor) degrades above 1024 wide -> split the wide
# chunks for the mul only
_QW = 1024


def _build_nc(mm_dtype_name="float16", warmup_full=8, warmup_short=14):
    import concourse.bass as bass
    import concourse.mybir as mybir
    from concourse.tile import TileContext
    from concourse.bass import _add_dep_helper

    dt = mybir.dt
    mm_dt = getattr(dt, mm_dtype_name)
    grid = _grid_f32()

    nc = bass.Bass()

    xt = nc.dram_tensor("xt", [IN_DIM, BC], mm_dt, kind="ExternalInput")
    cw = nc.dram_tensor("cw", [NM * IN_DIM, OUT_DIM], mm_dt,
                        kind="ExternalInput")
    out = nc.dram_tensor("out", [BC, OUT_DIM], mm_dt,
                         kind="ExternalOutput")

    ACTF = mybir.ActivationFunctionType
    ALU = mybir.AluOpType

    with TileContext(nc) as tc:
        with tc.tile_pool(name="main", bufs=1) as pool, \
             tc.tile_pool(name="psum", bufs=1, space="PSUM") as psum_pool:
            X = pool.tile([P, W_TOT], mm_dt, tag="X")
            CW = pool.tile([P, NM * NIC * OUT_DIM], mm_dt, tag="CW")

            # PE warmup: matmuls over a zeroed dummy tile into a spare
            # psum bank so the PE clock is at full speed when real
            # matmuls arrive (it ramps over ~3.5us of continuous work).
            dumb = pool.tile([P, OUT_DIM], mm_dt, tag="dumb")
            dpsum = psum_pool.tile([P, OUT_DIM], dt.float32, tag="dps",
                                   name="dps")
            nc.gpsimd.memset(dumb[:], 0.0)
            for _ in range(warmup_full):
                nc.tensor.matmul(dpsum[:], dumb[:, 0:P], dumb[:],
                                 start=True, stop=True)
            for _ in range(warmup_short):
                nc.tensor.matmul(dpsum[:, 0:P], dumb[:, 0:P],
                                 dumb[:, 0:P], start=True, stop=True)

            # ---- input DMAs. Layout is partition-major (k = p*NIC+t) on
            # both sides of the matmul, so each W-group DMA is 128
            # contiguous descriptors. X lands in a small ic0 piece (sync
            # queue) + the rest (scalar queue); W groups chain behind
            # them with forced semaphore deps so the early tensors get
            # the full DMA bandwidth in consumption order.
            xt_r = xt.rearrange("(p t) b -> p t b", p=P)

            def dma_x(eng, t0, t1):
                return getattr(nc, eng).dma_start(
                    out=X[:, t0 * BC:t1 * BC]
                        .rearrange("p (t b) -> p t b", t=t1 - t0),
                    in_=xt_r[:, t0:t1, :])

            def dma_w(m, t0, t1, eng="sync"):
                grp = cw[m * IN_DIM:(m + 1) * IN_DIM, :] \
                    .rearrange("(p t) j -> p t j", p=P)
                return getattr(nc, eng).dma_start(
                    out=CW[:, (m * NIC + t0) * OUT_DIM:
                           (m * NIC + t1) * OUT_DIM]
                        .rearrange("p (t j) -> p t j", t=t1 - t0),
                    in_=grp[:, t0:t1, :])

            xpA = dma_x("sync", 0, 1)      # ic0: gates the first chunk
            xpB = dma_x("scalar", 1, NIC)  # ic1..3
            dma_w(0, 0, 1, eng="gpsimd")   # W(m0, t0): first matmuls
            # two parallel W chains staggered behind the X pieces so the
            # early tensors get the full DMA bandwidth
            chain_a, chain_b = xpA, xpB
            for i, (m, t0, t1) in enumerate([(0, 1, NIC)]
                                            + [(m, 0, NIC)
                                               for m in range(1, NM)]):
                eng = "gpsimd" if m == NM - 1 else "sync"
                wd = dma_w(m, t0, t1, eng=eng)
                prev = chain_a if i % 2 == 0 else chain_b
                _add_dep_helper(wd.ins, prev.ins, sync=True,
                                reason="stagger W DMAs behind X/previous")
                if i % 2 == 0:
                    chain_a = wd
                else:
                    chain_b = wd

            def w_tile(m, ic):
                o = (m * NIC + ic) * OUT_DIM
                return CW[:, o:o + OUT_DIM]

            # ---- features: r = max(x-g, 0) [DVE], s = r^2 [ACT],
            # q = fp16(r*s) [DVE]. Chunked so the first q exists ASAP.
            R = [pool.tile([P, W_TOT], dt.float32, tag=f"r{m}",
                           name=f"r{m}") for m in range(NM)]
            S = [pool.tile([P, W_TOT], dt.float32, tag=f"s{m}",
                           name=f"s{m}") for m in range(NM)]
            Q = [pool.tile([P, W_TOT], mm_dt, tag=f"q{m}",
                           name=f"q{m}") for m in range(NM)]

            prev_dve = [None]

            def dve_order(inst):
                if prev_dve[0] is not None:
                    _add_dep_helper(inst.ins, prev_dve[0].ins, sync=False,
                                    reason="DVE consumption order")
                prev_dve[0] = inst
                return inst

            for m in range(NM):
                gm = float(grid[m])
                for (c0, c1) in _CHUNKS[m]:
                    dve_order(nc.vector.tensor_scalar(
                        R[m][:, c0:c1], X[:, c0:c1], gm, 0.0,
                        ALU.subtract, ALU.max))
                    nc.scalar.activation(S[m][:, c0:c1], R[m][:, c0:c1],
                                         ACTF.Square)
                    for q0 in range(c0, c1, _QW):
                        q1 = min(q0 + _QW, c1)
                        dve_order(nc.vector.tensor_mul(
                            Q[m][:, q0:q1], R[m][:, q0:q1],
                            S[m][:, q0:q1]))

            # ---- matmuls. m0/m1 iterate ic-outer (chunk-gated start);
            # later m's bc-outer so each psum finishes early in the m4
            # round and evictions overlap the tail.
            psums = [psum_pool.tile([P, OUT_DIM], dt.float32, tag=f"ps{b}",
                                    name=f"ps{b}")
                     for b in range(NBC)]
            O = pool.tile([P, NBC * OUT_DIM], mm_dt, tag="O")
            out_dmas = []

            def mm(m, bc, ic):
                kc = m * NIC + ic
                lhsT = Q[m][:, ic * BC + bc * P: ic * BC + (bc + 1) * P]
                nc.tensor.matmul(psums[bc][:], lhsT, w_tile(m, ic),
                                 start=(kc == 0),
                                 stop=(kc == NM * NIC - 1))

            for m in range(NM):
                if m in (0, 1):
                    for ic in range(NIC):
                        for bc in range(NBC):
                            mm(m, bc, ic)
                else:
                    for bc in range(NBC):
                        for ic in range(NIC):
                            mm(m, bc, ic)
                        if m == NM - 1:
                            nc.scalar.activation(
                                O[:, bc * OUT_DIM:(bc + 1) * OUT_DIM],
                                psums[bc][:], ACTF.Copy)
                            if bc in (1, NBC - 1):
                                # output drains in two chained halves on
                                # the scalar queue: the first needs no
                                # waits (evictions precede it in FIFO),
                                # the second waits only the first's
                                # completion, so the final drain's single
                                # wait covers both.
                                b0, nb = (0, 2) if bc == 1 else (2, 2)
                                od = nc.scalar.dma_start(
                                    out=out[b0 * P:(b0 + nb) * P, :]
                                        .rearrange("(c p) j -> p c j", p=P),
                                    in_=O[:, b0 * OUT_DIM:
                                          (b0 + nb) * OUT_DIM]
                                        .rearrange("p (c j) -> p c j", c=nb))
                                out_dmas.append(od)

    _strip_waits(nc, out_dmas)
    return nc


def _strip_waits(nc, out_dmas):
    """Walrus allows one sync wait per instruction (the final drain takes
    a few). Strip the provably redundant waits:
     - same-engine waits (engines are in-order FIFOs),
     - waits dominated by an earlier same-engine instruction's wait on
       the same semaphore with >= target value (FIFO order covers them),
     - DMASW same-queue WAR waits on DMA copies,
     - the final drain keeps only the last out-DMA's update sems.
    """
    import re
    eng2sem = {"EngineType.DVE": "DVE_",
               "EngineType.Activation": "Activation_",
               "EngineType.Pool": "Pool_",
               "EngineType.PE": "PE_",
               "EngineType.SP": "SP_"}
    # monotonic data-dep sems only: barrier sems reset/decrement, so
    # value-domination logic must never touch them
    _mono = re.compile(r"^(DVE|Activation|Pool|PE|SP)_\d+$"
                       r"|^DMA(HW|SW)\d+_\d+$")

    def _wait_val(w):
        return w.wait_value if w.wait_value is not None else -1

    # both out DMAs ride the same queue (in-order rings), so the last
    # one's completion implies the first's: the drain needs only the
    # last od's sem (walrus allows a single sync wait, even on drains)
    out_sems = set()
    if out_dmas:
        osi = out_dmas[-1].ins.sync_info
        for u in (osi.on_update if osi else []):
            if u.ant_name:
                out_sems.add(u.ant_name)

    bad = []
    for blk in nc.m.functions[0].blocks:
        covered = {}   # (engine, sem) -> max awaited value
        for inst in blk.instructions:
            si = inst.sync_info
            eng = str(inst.engine)
            if si is None or not si.on_wait:
                continue
            iname = type(inst).__name__
            pref = eng2sem.get(eng)
            if iname == "InstDMACopy":
                # DMA data movement is async w.r.t. the issuing engine's
                # pipeline: same-engine waits must be KEPT (the trigger
                # being later in the FIFO does not mean the producer op
                # has retired). Only same-queue WAR DMASW waits go.
                keep = list(si.on_wait)
                nq = [w for w in keep
                      if not (w.ant_name or "").startswith("DMASW")]
                if nq:
                    keep = nq
            else:
                keep = [w for w in si.on_wait
                        if pref is None
                        or not (w.ant_name or "").startswith(pref)]
            if iname == "InstDrain" and len(keep) > 1:
                sel = [w for w in keep if (w.ant_name or "") in out_sems]
                if sel:
                    keep = sel
            # drop waits dominated by an earlier same-engine wait
            # (monotonic named sems only)
            keep = [w for w in keep
                    if not (w.ant_name and _mono.match(w.ant_name))
                    or covered.get((eng, w.ant_name), -1) < _wait_val(w)]
            # same-semaphore waits collapse to the largest target value
            if len(keep) > 1:
                by_sem = {}
                for w in keep:
                    k = w.ant_name
                    if k not in by_sem or _wait_val(w) > _wait_val(by_sem[k]):
                        by_sem[k] = w
                keep = list(by_sem.values())
            for w in keep:
                if w.ant_name and _mono.match(w.ant_name):
                    k = (eng, w.ant_name)
                    if covered.get(k, -1) < _wait_val(w):
                        covered[k] = _wait_val(w)
            if len(keep) != len(si.on_wait):
                si.on_wait = keep
            if len(keep) > 1 and iname not in ("InstDrain",):
                bad.append((inst.name, iname,
                            [(w.ant_name, _wait_val(w)) for w in keep]))
    assert not bad, f"many-wait instructions remain: {bad}"


def _prep_weights(spline_coeff, spline_scaling):
    # C'[m,i,j] = (1/(6h^3)) * sum_g w[m-g] * coeff[i,j,g] * scaling[i,j]
    h = 2.0 / GRID_SIZE
    c = (spline_coeff.astype(np.float64)
         * spline_scaling.astype(np.float64)[:, :, None])  # [i, j, g]
    cp = np.zeros((NM, IN_DIM, OUT_DIM), np.float64)
    for m in range(NM):
        for g in range(max(0, m - 4), m + 1):
            cp[m] += _W_BINOM[m - g] * c[:, :, g]
    cp *= 1.0 / (6.0 * h ** 3)
    return np.ascontiguousarray(
        cp.reshape(NM * IN_DIM, OUT_DIM).astype(np.float32))


def _np_mm_dtype(mm_dtype_name):
    if mm_dtype_name == "float32":
        return np.float32
    if mm_dtype_name == "float16":
        return np.float16
    if mm_dtype_name == "bfloat16":
        import ml_dtypes
        return ml_dtypes.bfloat16
    raise ValueError(mm_dtype_name)


def _run(inputs, trace=False, mm_dtype_name="float16"):
    from concourse.bass_utils import run_bass_kernel_spmd

    key = mm_dtype_name
    if key not in _cached:
        _cached[key] = _build_nc(mm_dtype_name)
    nc = _cached[key]

    x = np.asarray(inputs["x"], np.float32)
    cw = _prep_weights(np.asarray(inputs["spline_coeff"]),
                       np.asarray(inputs["spline_scaling"]))
    cw = np.ascontiguousarray(cw.astype(_np_mm_dtype(mm_dtype_name)))
    in_maps = []
    for c in range(N_CORES):
        xc = np.ascontiguousarray(
            x[c * BC:(c + 1) * BC, :].T.astype(_np_mm_dtype(mm_dtype_name)))
        in_maps.append({"xt": xc, "cw": cw})
    res = run_bass_kernel_spmd(nc, in_maps, list(range(N_CORES)),
                               trace=trace)
    outp = np.concatenate([res.results[c]["out"] for c in range(N_CORES)],
                          axis=0).astype(np.float32)
    return outp, res


def kernel(**inputs):
    outp, _ = _run(inputs, trace=False)
    return outp


# revision 12
# speedup vs baseline: 1.4106x; 1.4106x over previous
"""KAN expert kernel for Trainium2 (8 NeuronCores, data-parallel over batch).

Math: out[b,j] = sum_{i,g} basis_g(x[b,i]) * coeff[i,j,g] * scaling[i,j]
with cubic B-spline basis on the uniform extended grid g_m = -1 + 0.4*m.

Key identity (truncated powers): for the uniform grid, the basis is the
cardinal cubic B-spline, basis_g(x) = (1/(6h^3)) * sum_{r=0..4} w_r *
relu(x - g_{g+r})^3 with w = [1,-4,6,-4,1]. Since x in [-1,1) only
relu-features m=0..4 are nonzero, and the (linear) binomial combine is
folded into the weights on the host:
    C'[m,i,j] = (1/(6h^3)) * sum_g w_{m-g} * coeff[i,j,g] * scaling[i,j]
so each core computes Q_m = relu(x - g_m)^3 (m=0..4) and a
[512b x 2560k] @ [2560k x 512j] fp16 matmul accumulated in fp32 PSUM.

Precision: the truncated-power split cancels heavily, so the matmul
INPUTS need >= 10 mantissa bits: fp16 passes (7.8e-3 rel vs the 2e-2
gate) IFF the features are computed in fp32 and rounded to fp16 once:
    r_m = max(x - g_m, 0)     (DVE tensor_scalar, fp32)
    s_m = Square(r_m)         (ACT, fp32; == (x-g)^2 wherever r>0,
                               and q=0 elsewhere anyway -> exact)
    q_m = fp16(r_m * s_m)     (DVE tensor_mul, single rounding)
Using Square(r) instead of Square(x - g) needs NO bias constants, so
the kernel has no pre-TileContext memsets/barrier: the measured window
(first useful instruction -> teardown end) starts at Bass's builtin
const memsets and the X DMA issues ~0.9us earlier than with the
const+barrier prologue.

Schedule (measured on HW):
 - X lands in two pieces: a small ic0 piece (sync queue) that gates the
   first feature chunk, and the rest (scalar queue). W groups chain
   behind them on two queues so early tensors get full DMA bandwidth.
 - first feature chunks are 512 wide to minimize the X->first-MM
   latency; later chunks 1024/2048 (cheaper per element).
 - LDWEIGHTS+MATMUL pairs sustain ~216ns/MM when fed; each half carries
   at most one sync wait (q-producer on LDWEIGHTS, W-arrival DMA on
   MATMUL). PE declocks 2x if it idles >3.4us -> dummy warmup matmuls
   run while the DMAs land.
 - a generic wait-domination pass strips every sync wait already
   covered by an earlier same-engine wait (engines are in-order FIFOs),
   leaving <=1 sync wait per instruction for walrus.
"""

import numpy as np

BATCH = 4096
IN_DIM = 512
OUT_DIM = 512
GRID_SIZE = 5
K = 3
N_CORES = 8
P = 128
NM = 5                      # relu^3 feature channels
BC = BATCH // N_CORES       # 512 batch rows per core
NIC = IN_DIM // P           # 4 input-dim chunks
NBC = BC // P               # 4 batch chunks (psum tiles)
W_TOT = NIC * BC            # 2048 feature columns per tile

_W_BINOM = np.array([1.0, -4.0, 6.0, -4.0, 1.0])

_cached = {}


def _grid_f32():
    h = 2.0 / GRID_SIZE
    return np.float32(-1.0 + h * np.arange(GRID_SIZE + 2 * K + 1))


# per-m column chunking of the [128, 2048] feature space (ic-major, so
# [0:512] is exactly the ic0 block the first matmuls need).  R chunks
# start small (latency) then go full width (throughput); Q (3-stream
# tensor_tensor) degrades above 1024 so the mul is always <=1024 wide.
_R_CHUNKS = {
    0: [(0, 512), (512, 1024), (1024, 2048)],
    1: [(0, 2048)],
    2: [(0, 2048)],
    3: [(0, 2048)],
    4: [(0, 2048)],
}
_Q_CHUNKS = {
    0: [(0, 512), (512, 1024), (1024, 2048)],
    1: [(0, 1024), (1024, 2048)],
    2: [(0, 1024), (1024, 2048)],
    3: [(0, 1024), (1024, 2048)],
    4: [(0, 1024), (1024, 2048)],
}
# m-channels whose whole feature chain runs in fp16 (2x DVE rate).
# Precision (host-emulated on the real inputs): {} -> 7.84e-3,
# {3,4} -> 7.86e-3, {2,3,4} -> 1.18e-2, {1,..} -> 2.05e-2 (fails).
_FP16_MS = {3, 4}


def _build_nc(mm_dtype_name="float16", warmup_full=8, warmup_short=14):
    import concourse.bass as bass
    import concourse.mybir as mybir
    from concourse.tile import TileContext
    from concourse.bass import _add_dep_helper

    dt = mybir.dt
    mm_dt = getattr(dt, mm_dtype_name)
    grid = _grid_f32()

    nc = bass.Bass()

    xt = nc.dram_tensor("xt", [IN_DIM, BC], mm_dt, kind="ExternalInput")
    cw = nc.dram_tensor("cw", [NM * IN_DIM, OUT_DIM], mm_dt,
                        kind="ExternalInput")
    out = nc.dram_tensor("out", [BC, OUT_DIM], mm_dt,
                         kind="ExternalOutput")

    ACTF = mybir.ActivationFunctionType
    ALU = mybir.AluOpType

    with TileContext(nc) as tc:
        with tc.tile_pool(name="main", bufs=1) as pool, \
             tc.tile_pool(name="psum", bufs=1, space="PSUM") as psum_pool:
            X = pool.tile([P, W_TOT], mm_dt, tag="X")
            CW = pool.tile([P, NM * NIC * OUT_DIM], mm_dt, tag="CW")

            # ---- input DMAs. Layout is partition-major (k = p*NIC+t) on
            # both sides of the matmul, so each W-group DMA is 128
            # contiguous descriptors.  The small ic0 X piece goes FIRST
            # on the gpsimd/SWDGE queue (the Pool sequencer finishes its
            # framework preamble ~0.6us before SP, and SWDGE has lower
            # trigger->data latency), so the feature chain starts ASAP.
            # W groups chain behind the X pieces on two queues so early
            # tensors get the full DMA bandwidth in consumption order.
            xt_r = xt.rearrange("(p t) b -> p t b", p=P)

            def dma_x(eng, t0, t1):
                return getattr(nc, eng).dma_start(
                    out=X[:, t0 * BC:t1 * BC]
                        .rearrange("p (t b) -> p t b", t=t1 - t0),
                    in_=xt_r[:, t0:t1, :])

            def dma_w(m, t0, t1, eng="sync"):
                grp = cw[m * IN_DIM:(m + 1) * IN_DIM, :] \
                    .rearrange("(p t) j -> p t j", p=P)
                return getattr(nc, eng).dma_start(
                    out=CW[:, (m * NIC + t0) * OUT_DIM:
                           (m * NIC + t1) * OUT_DIM]
                        .rearrange("p (t j) -> p t j", t=t1 - t0),
                    in_=grp[:, t0:t1, :])

            xpA = dma_x("gpsimd", 0, 1)    # ic0: gates the first chunk
            dma_w(0, 0, 1, eng="gpsimd")   # W(m0, t0): first matmuls
            xpB = dma_x("sync", 1, NIC)    # ic1..3

            # PE warmup: matmuls over a zeroed dummy tile into a spare
            # psum bank so the PE clock is at full speed when real
            # matmuls arrive (it ramps over ~3.5us of continuous work).
            dumb = pool.tile([P, OUT_DIM], mm_dt, tag="dumb")
            dpsum = psum_pool.tile([P, OUT_DIM], dt.float32, tag="dps",
                                   name="dps")
            nc.gpsimd.memset(dumb[:], 0.0)
            for _ in range(warmup_full):
                nc.tensor.matmul(dpsum[:], dumb[:, 0:P], dumb[:],
                                 start=True, stop=True)
            for _ in range(warmup_short):
                nc.tensor.matmul(dpsum[:, 0:P], dumb[:, 0:P],
                                 dumb[:, 0:P], start=True, stop=True)

            # two parallel W chains staggered behind the X pieces so the
            # early tensors get the full DMA bandwidth
            chain_a, chain_b = xpA, xpB
            for i, (m, t0, t1) in enumerate([(0, 1, NIC)]
                                            + [(m, 0, NIC)
                                               for m in range(1, NM)]):
                eng = "sync" if i % 2 == 0 else "scalar"
                wd = dma_w(m, t0, t1, eng=eng)
                prev = chain_a if i % 2 == 0 else chain_b
                _add_dep_helper(wd.ins, prev.ins, sync=True,
                                reason="stagger W DMAs behind X/previous")
                if i % 2 == 0:
                    chain_a = wd
                else:
                    chain_b = wd

            def w_tile(m, ic):
                o = (m * NIC + ic) * OUT_DIM
                return CW[:, o:o + OUT_DIM]

            # ---- features: r = max(x-g, 0) [DVE], s = r^2 [ACT],
            # q = r*s rounded once [DVE].  m in _FP16_MS runs the whole
            # chain in fp16 (2x DVE rate; precision verified on host).
            # Software-pipelined emission: R(m+1) is queued on DVE
            # before Q(m) so DVE never stalls on ACT's square.
            def ft_dt(m):
                return mm_dt if m in _FP16_MS else dt.float32

            R = [pool.tile([P, W_TOT], ft_dt(m), tag=f"r{m}",
                           name=f"r{m}") for m in range(NM)]
            S = [pool.tile([P, W_TOT], ft_dt(m), tag=f"s{m}",
                           name=f"s{m}") for m in range(NM)]
            Q = [pool.tile([P, W_TOT], mm_dt, tag=f"q{m}",
                           name=f"q{m}") for m in range(NM)]

            prev_dve = [None]

            def dve_order(inst):
                if prev_dve[0] is not None:
                    _add_dep_helper(inst.ins, prev_dve[0].ins, sync=False,
                                    reason="DVE consumption order")
                prev_dve[0] = inst
                return inst

            def emit_r(m):
                gm = float(grid[m])
                for (c0, c1) in _R_CHUNKS[m]:
                    dve_order(nc.vector.tensor_scalar(
                        R[m][:, c0:c1], X[:, c0:c1], gm, 0.0,
                        ALU.subtract, ALU.max))
                for (c0, c1) in _Q_CHUNKS[m]:
                    nc.scalar.activation(S[m][:, c0:c1], R[m][:, c0:c1],
                                         ACTF.Square)

            def emit_q(m):
                for (c0, c1) in _Q_CHUNKS[m]:
                    dve_order(nc.vector.tensor_mul(
                        Q[m][:, c0:c1], R[m][:, c0:c1], S[m][:, c0:c1]))

            # DVE order: R0a R0b Q0a R0c Q0b R1 Q0c R2 Q1 R3 Q2 R4 Q3 Q4
            gm0 = float(grid[0])
            for i, (c0, c1) in enumerate(_R_CHUNKS[0]):
                dve_order(nc.vector.tensor_scalar(
                    R[0][:, c0:c1], X[:, c0:c1], gm0, 0.0,
                    ALU.subtract, ALU.max))
                nc.scalar.activation(S[0][:, c0:c1], R[0][:, c0:c1],
                                     ACTF.Square)
                if i >= 1:
                    p0, p1 = _Q_CHUNKS[0][i - 1]
                    dve_order(nc.vector.tensor_mul(
                        Q[0][:, p0:p1], R[0][:, p0:p1], S[0][:, p0:p1]))
            emit_r(1)
            p0, p1 = _Q_CHUNKS[0][-1]
            dve_order(nc.vector.tensor_mul(
                Q[0][:, p0:p1], R[0][:, p0:p1], S[0][:, p0:p1]))
            for m in range(2, NM):
                emit_r(m)
                emit_q(m - 1)
            emit_q(NM - 1)

            # ---- matmuls, consumed in Q-production order (m, ic) with
            # bc innermost: the stream never stalls mid-flight as long
            # as production stays ahead.  The m4/ic3 round interleaves
            # the psum evictions + output DMAs.
            psums = [psum_pool.tile([P, OUT_DIM], dt.float32, tag=f"ps{b}",
                                    name=f"ps{b}")
                     for b in range(NBC)]
            O = pool.tile([P, NBC * OUT_DIM], mm_dt, tag="O")
            out_dmas = []

            def mm(m, bc, ic):
                kc = m * NIC + ic
                lhsT = Q[m][:, ic * BC + bc * P: ic * BC + (bc + 1) * P]
                nc.tensor.matmul(psums[bc][:], lhsT, w_tile(m, ic),
                                 start=(kc == 0),
                                 stop=(kc == NM * NIC - 1))

            for m in range(NM):
                for ic in range(NIC):
                    last = (m == NM - 1 and ic == NIC - 1)
                    for bc in range(NBC):
                        mm(m, bc, ic)
                        if last:
                            nc.scalar.activation(
                                O[:, bc * OUT_DIM:(bc + 1) * OUT_DIM],
                                psums[bc][:], ACTF.Copy)
                            if bc in (1, NBC - 1):
                                # two output halves on the scalar queue
                                # (same rings -> in-order completion, so
                                # the final drain waits only the last)
                                b0, nb = (0, 2) if bc == 1 else (2, 2)
                                od = nc.scalar.dma_start(
                                    out=out[b0 * P:(b0 + nb) * P, :]
                                        .rearrange("(c p) j -> p c j", p=P),
                                    in_=O[:, b0 * OUT_DIM:
                                          (b0 + nb) * OUT_DIM]
                                        .rearrange("p (c j) -> p c j", c=nb))
                                out_dmas.append(od)

    _strip_waits(nc, out_dmas)
    return nc


def _strip_waits(nc, out_dmas):
    """Walrus allows one sync wait per instruction (the final drain takes
    a few). Strip the provably redundant waits:
     - same-engine waits (engines are in-order FIFOs),
     - waits dominated by an earlier same-engine instruction's wait on
       the same semaphore with >= target value (FIFO order covers them),
     - DMASW same-queue WAR waits on DMA copies,
     - the final drain keeps only the last out-DMA's update sems.
    """
    import re
    eng2sem = {"EngineType.DVE": "DVE_",
               "EngineType.Activation": "Activation_",
               "EngineType.Pool": "Pool_",
               "EngineType.PE": "PE_",
               "EngineType.SP": "SP_"}
    # monotonic data-dep sems only: barrier sems reset/decrement, so
    # value-domination logic must never touch them
    _mono = re.compile(r"^(DVE|Activation|Pool|PE|SP)_\d+$"
                       r"|^DMA(HW|SW)\d+_\d+$")

    def _wait_val(w):
        return w.wait_value if w.wait_value is not None else -1

    # both out DMAs ride the same queue (in-order rings), so the last
    # one's completion implies the first's: the drain needs only the
    # last od's sem (walrus allows a single sync wait, even on drains)
    out_sems = set()
    if out_dmas:
        osi = out_dmas[-1].ins.sync_info
        for u in (osi.on_update if osi else []):
            if u.ant_name:
                out_sems.add(u.ant_name)

    bad = []
    for blk in nc.m.functions[0].blocks:
        covered = {}   # (engine, sem) -> max awaited value
        for inst in blk.instructions:
            si = inst.sync_info
            eng = str(inst.engine)
            if si is None or not si.on_wait:
                continue
            iname = type(inst).__name__
            pref = eng2sem.get(eng)
            if iname == "InstDMACopy":
                # DMA data movement is async w.r.t. the issuing engine's
                # pipeline: same-engine waits must be KEPT (the trigger
                # being later in the FIFO does not mean the producer op
                # has retired). Only same-queue WAR DMASW waits go.
                keep = list(si.on_wait)
                nq = [w for w in keep
                      if not (w.ant_name or "").startswith("DMASW")]
                if nq:
                    keep = nq
            else:
                keep = [w for w in si.on_wait
                        if pref is None
                        or not (w.ant_name or "").startswith(pref)]
            if iname == "InstDrain" and len(keep) > 1:
                sel = [w for w in keep if (w.ant_name or "") in out_sems]
                if sel:
                    keep = sel
            # drop waits dominated by an earlier same-engine wait
            # (monotonic named sems only)
            keep = [w for w in keep
                    if not (w.ant_name and _mono.match(w.ant_name))
                    or covered.get((eng, w.ant_name), -1) < _wait_val(w)]
            # same-semaphore waits collapse to the largest target value
            if len(keep) > 1:
                by_sem = {}
                for w in keep:
                    k = w.ant_name
                    if k not in by_sem or _wait_val(w) > _wait_val(by_sem[k]):
                        by_sem[k] = w
                keep = list(by_sem.values())
            for w in keep:
                if w.ant_name and _mono.match(w.ant_name):
                    k = (eng, w.ant_name)
                    if covered.get(k, -1) < _wait_val(w):
                        covered[k] = _wait_val(w)
            if len(keep) != len(si.on_wait):
                si.on_wait = keep
            if len(keep) > 1 and iname not in ("InstDrain",):
                bad.append((inst.name, iname,
                            [(w.ant_name, _wait_val(w)) for w in keep]))
    assert not bad, f"many-wait instructions remain: {bad}"


def _prep_weights(spline_coeff, spline_scaling):
    # C'[m,i,j] = (1/(6h^3)) * sum_g w[m-g] * coeff[i,j,g] * scaling[i,j]
    h = 2.0 / GRID_SIZE
    c = (spline_coeff.astype(np.float64)
         * spline_scaling.astype(np.float64)[:, :, None])  # [i, j, g]
    cp = np.zeros((NM, IN_DIM, OUT_DIM), np.float64)
    for m in range(NM):
        for g in range(max(0, m - 4), m + 1):
            cp[m] += _W_BINOM[m - g] * c[:, :, g]
    cp *= 1.0 / (6.0 * h ** 3)
    return np.ascontiguousarray(
        cp.reshape(NM * IN_DIM, OUT_DIM).astype(np.float32))


def _np_mm_dtype(mm_dtype_name):
    if mm_dtype_name == "float32":
        return np.float32
    if mm_dtype_name == "float16":
        return np.float16
    if mm_dtype_name == "bfloat16":
        import ml_dtypes
        return ml_dtypes.bfloat16
    raise ValueError(mm_dtype_name)


def _run(inputs, trace=False, mm_dtype_name="float16"):
    from concourse.bass_utils import run_bass_kernel_spmd

    key = mm_dtype_name
    if key not in _cached:
        _cached[key] = _build_nc(mm_dtype_name)
    nc = _cached[key]

    x = np.asarray(inputs["x"], np.float32)
    cw = _prep_weights(np.asarray(inputs["spline_coeff"]),
                       np.asarray(inputs["spline_scaling"]))
    cw = np.ascontiguousarray(cw.astype(_np_mm_dtype(mm_dtype_name)))
    in_maps = []
    for c in range(N_CORES):
        xc = np.ascontiguousarray(
            x[c * BC:(c + 1) * BC, :].T.astype(_np_mm_dtype(mm_dtype_name)))
        in_maps.append({"xt": xc, "cw": cw})
    res = run_bass_kernel_spmd(nc, in_maps, list(range(N_CORES)),
                               trace=trace)
    outp = np.concatenate([res.results[c]["out"] for c in range(N_CORES)],
                          axis=0).astype(np.float32)
    return outp, res


def kernel(**inputs):
    outp, _ = _run(inputs, trace=False)
    return outp


# revision 14
# speedup vs baseline: 1.4713x; 1.0430x over previous
"""KAN expert kernel for Trainium2 (8 NeuronCores, data-parallel over batch).

Math: out[b,j] = sum_{i,g} basis_g(x[b,i]) * coeff[i,j,g] * scaling[i,j]
with cubic B-spline basis on the uniform extended grid g_m = -1 + 0.4*m.

Key identity (truncated powers): for the uniform grid, the basis is the
cardinal cubic B-spline, basis_g(x) = (1/(6h^3)) * sum_{r=0..4} w_r *
relu(x - g_{g+r})^3 with w = [1,-4,6,-4,1]. Since x in [-1,1) only
relu-features m=0..4 are nonzero, and the (linear) binomial combine is
folded into the weights on the host:
    C'[m,i,j] = (1/(6h^3)) * sum_g w_{m-g} * coeff[i,j,g] * scaling[i,j]
so each core computes Q_m = relu(x - g_m)^3 (m=0..4) and a
[512b x 2560k] @ [2560k x 512j] fp16 matmul accumulated in fp32 PSUM.

Precision: the truncated-power split cancels heavily, so the matmul
INPUTS need >= 10 mantissa bits: fp16 passes (7.8e-3 rel vs the 2e-2
gate) IFF the features are computed in fp32 and rounded to fp16 once:
    r_m = max(x - g_m, 0)     (DVE tensor_scalar, fp32)
    s_m = Square(r_m)         (ACT, fp32; == (x-g)^2 wherever r>0,
                               and q=0 elsewhere anyway -> exact)
    q_m = fp16(r_m * s_m)     (DVE tensor_mul, single rounding)
Using Square(r) instead of Square(x - g) needs NO bias constants, so
the kernel has no pre-TileContext memsets/barrier: the measured window
(first useful instruction -> teardown end) starts at Bass's builtin
const memsets and the X DMA issues ~0.9us earlier than with the
const+barrier prologue.

Schedule (measured on HW):
 - X lands in two pieces: a small ic0 piece (sync queue) that gates the
   first feature chunk, and the rest (scalar queue). W groups chain
   behind them on two queues so early tensors get full DMA bandwidth.
 - first feature chunks are 512 wide to minimize the X->first-MM
   latency; later chunks 1024/2048 (cheaper per element).
 - LDWEIGHTS+MATMUL pairs sustain ~216ns/MM when fed; each half carries
   at most one sync wait (q-producer on LDWEIGHTS, W-arrival DMA on
   MATMUL). PE declocks 2x if it idles >3.4us -> dummy warmup matmuls
   run while the DMAs land.
 - a generic wait-domination pass strips every sync wait already
   covered by an earlier same-engine wait (engines are in-order FIFOs),
   leaving <=1 sync wait per instruction for walrus.
"""

import numpy as np

BATCH = 4096
IN_DIM = 512
OUT_DIM = 512
GRID_SIZE = 5
K = 3
N_CORES = 8
P = 128
NM = 5                      # relu^3 feature channels
BC = BATCH // N_CORES       # 512 batch rows per core
NIC = IN_DIM // P           # 4 input-dim chunks
NBC = BC // P               # 4 batch chunks (psum tiles)
W_TOT = NIC * BC            # 2048 feature columns per tile

_W_BINOM = np.array([1.0, -4.0, 6.0, -4.0, 1.0])

_cached = {}


def _grid_f32():
    h = 2.0 / GRID_SIZE
    return np.float32(-1.0 + h * np.arange(GRID_SIZE + 2 * K + 1))


# per-m column chunking of the [128, 2048] feature space (ic-major, so
# [0:512] is exactly the ic0 block the first matmuls need).  R chunks
# start small (latency) then go full width (throughput); Q (3-stream
# tensor_tensor) degrades above 1024 so the mul is always <=1024 wide.
_R_CHUNKS = {
    0: [(0, 512), (512, 1024), (1024, 2048)],
    1: [(0, 2048)],
    2: [(0, 2048)],
    3: [(0, 2048)],
    4: [(0, 2048)],
}
_Q_CHUNKS = {
    0: [(0, 512), (512, 1024), (1024, 2048)],
    1: [(0, 1024), (1024, 2048)],
    2: [(0, 1024), (1024, 2048)],
    3: [(0, 1024), (1024, 2048)],
    4: [(0, 1024), (1024, 2048)],
}
# m-channels whose whole feature chain runs in fp16 (2x DVE rate).
# Precision (host-emulated on the real inputs): {} -> 7.84e-3,
# {3,4} -> 7.86e-3, {2,3,4} -> 1.18e-2, {1,..} -> 2.05e-2 (fails).
_FP16_MS = {3, 4}


def _build_nc(mm_dtype_name="float16", warmup_full=8, warmup_short=14):
    import concourse.bass as bass
    import concourse.mybir as mybir
    from concourse.tile import TileContext
    from concourse.bass import _add_dep_helper

    dt = mybir.dt
    mm_dt = getattr(dt, mm_dtype_name)
    grid = _grid_f32()

    nc = bass.Bass()

    xt = nc.dram_tensor("xt", [IN_DIM, BC], mm_dt, kind="ExternalInput")
    cw = nc.dram_tensor("cw", [NM * IN_DIM, OUT_DIM], mm_dt,
                        kind="ExternalInput")
    out = nc.dram_tensor("out", [BC, OUT_DIM], mm_dt,
                         kind="ExternalOutput")

    ACTF = mybir.ActivationFunctionType
    ALU = mybir.AluOpType

    with TileContext(nc) as tc:
        with tc.tile_pool(name="main", bufs=1) as pool, \
             tc.tile_pool(name="psum", bufs=1, space="PSUM") as psum_pool:
            X = pool.tile([P, W_TOT], mm_dt, tag="X")
            CW = pool.tile([P, NM * NIC * OUT_DIM], mm_dt, tag="CW")

            # ---- input DMAs. Layout is partition-major (k = p*NIC+t) on
            # both sides of the matmul, so each W-group DMA is 128
            # contiguous descriptors.  The small ic0 X piece goes FIRST
            # on the gpsimd/SWDGE queue (the Pool sequencer finishes its
            # framework preamble ~0.6us before SP, and SWDGE has lower
            # trigger->data latency), so the feature chain starts ASAP.
            # W groups chain behind the X pieces on two queues so early
            # tensors get the full DMA bandwidth in consumption order.
            xt_r = xt.rearrange("(p t) b -> p t b", p=P)

            def dma_x(eng, t0, t1):
                return getattr(nc, eng).dma_start(
                    out=X[:, t0 * BC:t1 * BC]
                        .rearrange("p (t b) -> p t b", t=t1 - t0),
                    in_=xt_r[:, t0:t1, :])

            def dma_w(m, t0, t1, eng="sync"):
                grp = cw[m * IN_DIM:(m + 1) * IN_DIM, :] \
                    .rearrange("(p t) j -> p t j", p=P)
                return getattr(nc, eng).dma_start(
                    out=CW[:, (m * NIC + t0) * OUT_DIM:
                           (m * NIC + t1) * OUT_DIM]
                        .rearrange("p (t j) -> p t j", t=t1 - t0),
                    in_=grp[:, t0:t1, :])

            # Two priority-ordered chains, one per idle queue (gpsimd/
            # SWDGE + sync/HWDGE), so exactly two transfers share HBM
            # bandwidth at any time, in consumption order.  Neither the
            # Scalar nor Vector sequencer carries DMA triggers: a
            # trigger's chain-wait head-of-line blocks the whole
            # sequencer, which would stall the squares/muls behind it.
            xpA = dma_x("gpsimd", 0, 1)      # ic0: gates R0a

            # PE warmup: matmuls over a zeroed dummy tile into a spare
            # psum bank so the PE clock is at full speed when real
            # matmuls arrive (it ramps over ~3.5us of continuous work).
            # The memset sits between the XA trigger and the chained W
            # triggers on the Pool stream, so warmup starts early.
            dumb = pool.tile([P, OUT_DIM], mm_dt, tag="dumb")
            dpsum = psum_pool.tile([P, OUT_DIM], dt.float32, tag="dps",
                                   name="dps")
            nc.gpsimd.memset(dumb[:], 0.0)
            for _ in range(warmup_full):
                nc.tensor.matmul(dpsum[:], dumb[:, 0:P], dumb[:],
                                 start=True, stop=True)
            for _ in range(warmup_short):
                nc.tensor.matmul(dpsum[:, 0:P], dumb[:, 0:P],
                                 dumb[:, 0:P], start=True, stop=True)

            def chain(wd, prev):
                _add_dep_helper(wd.ins, prev.ins, sync=True,
                                reason="stagger DMAs in consumption order")
                return wd

            # chain 1 (gpsimd): XA -> W(m0,t0) -> W(m1) -> W(m3)
            c1 = chain(dma_w(0, 0, 1, eng="gpsimd"), xpA)
            c1 = chain(dma_w(1, 0, NIC, eng="gpsimd"), c1)
            chain(dma_w(3, 0, NIC, eng="gpsimd"), c1)
            # chain 2 (sync): ic1 -> ic2,3 -> W(m0,t1:4) -> W(m2) -> W(m4)
            c2 = dma_x("sync", 1, 2)         # ic1: gates R0b
            c2 = chain(dma_x("sync", 2, NIC), c2)
            c2 = chain(dma_w(0, 1, NIC, eng="sync"), c2)
            c2 = chain(dma_w(2, 0, NIC, eng="sync"), c2)
            chain(dma_w(4, 0, NIC, eng="sync"), c2)

            def w_tile(m, ic):
                o = (m * NIC + ic) * OUT_DIM
                return CW[:, o:o + OUT_DIM]

            # ---- features: r = max(x-g, 0) [DVE], s = r^2 [ACT],
            # q = r*s rounded once [DVE].  m in _FP16_MS runs the whole
            # chain in fp16 (2x DVE rate; precision verified on host).
            # Software-pipelined emission: R(m+1) is queued on DVE
            # before Q(m) so DVE never stalls on ACT's square.
            def ft_dt(m):
                return mm_dt if m in _FP16_MS else dt.float32

            R = [pool.tile([P, W_TOT], ft_dt(m), tag=f"r{m}",
                           name=f"r{m}") for m in range(NM)]
            S = [pool.tile([P, W_TOT], ft_dt(m), tag=f"s{m}",
                           name=f"s{m}") for m in range(NM)]
            Q = [pool.tile([P, W_TOT], mm_dt, tag=f"q{m}",
                           name=f"q{m}") for m in range(NM)]

            prev_dve = [None]

            def dve_order(inst):
                if prev_dve[0] is not None:
                    _add_dep_helper(inst.ins, prev_dve[0].ins, sync=False,
                                    reason="DVE consumption order")
                prev_dve[0] = inst
                return inst

            def emit_r(m):
                gm = float(grid[m])
                for (c0, c1) in _R_CHUNKS[m]:
                    dve_order(nc.vector.tensor_scalar(
                        R[m][:, c0:c1], X[:, c0:c1], gm, 0.0,
                        ALU.subtract, ALU.max))
                for (c0, c1) in _Q_CHUNKS[m]:
                    nc.scalar.activation(S[m][:, c0:c1], R[m][:, c0:c1],
                                         ACTF.Square)

            def emit_q(m):
                for (c0, c1) in _Q_CHUNKS[m]:
                    dve_order(nc.vector.tensor_mul(
                        Q[m][:, c0:c1], R[m][:, c0:c1], S[m][:, c0:c1]))

            # DVE order: R0a R0b Q0a R0c Q0b R1 Q0c R2 Q1 R3 Q2 R4 Q3 Q4
            gm0 = float(grid[0])
            for i, (c0, c1) in enumerate(_R_CHUNKS[0]):
                dve_order(nc.vector.tensor_scalar(
                    R[0][:, c0:c1], X[:, c0:c1], gm0, 0.0,
                    ALU.subtract, ALU.max))
                nc.scalar.activation(S[0][:, c0:c1], R[0][:, c0:c1],
                                     ACTF.Square)
                if i >= 1:
                    p0, p1 = _Q_CHUNKS[0][i - 1]
                    dve_order(nc.vector.tensor_mul(
                        Q[0][:, p0:p1], R[0][:, p0:p1], S[0][:, p0:p1]))
            emit_r(1)
            p0, p1 = _Q_CHUNKS[0][-1]
            dve_order(nc.vector.tensor_mul(
                Q[0][:, p0:p1], R[0][:, p0:p1], S[0][:, p0:p1]))
            for m in range(2, NM):
                emit_r(m)
                emit_q(m - 1)
            emit_q(NM - 1)

            # ---- matmuls, consumed in Q-production order (m, ic) with
            # bc innermost: the stream never stalls mid-flight as long
            # as production stays ahead.  The m4/ic3 round interleaves
            # the psum evictions + output DMAs.
            psums = [psum_pool.tile([P, OUT_DIM], dt.float32, tag=f"ps{b}",
                                    name=f"ps{b}")
                     for b in range(NBC)]
            O = pool.tile([P, NBC * OUT_DIM], mm_dt, tag="O")
            out_dmas = []

            def mm(m, bc, ic):
                kc = m * NIC + ic
                lhsT = Q[m][:, ic * BC + bc * P: ic * BC + (bc + 1) * P]
                nc.tensor.matmul(psums[bc][:], lhsT, w_tile(m, ic),
                                 start=(kc == 0),
                                 stop=(kc == NM * NIC - 1))

            for m in range(NM):
                for ic in range(NIC):
                    last = (m == NM - 1 and ic == NIC - 1)
                    for bc in range(NBC):
                        mm(m, bc, ic)
                        if last:
                            nc.scalar.activation(
                                O[:, bc * OUT_DIM:(bc + 1) * OUT_DIM],
                                psums[bc][:], ACTF.Copy)
                            if bc in (1, NBC - 1):
                                # two output halves on the sync queue
                                # (idle at the tail, and the trigger
                                # would head-of-line block the copies on
                                # scalar).  Same rings -> in-order
                                # completion, so the final drain waits
                                # only the last one's sem.
                                b0, nb = (0, 2) if bc == 1 else (2, 2)
                                od = nc.sync.dma_start(
                                    out=out[b0 * P:(b0 + nb) * P, :]
                                        .rearrange("(c p) j -> p c j", p=P),
                                    in_=O[:, b0 * OUT_DIM:
                                          (b0 + nb) * OUT_DIM]
                                        .rearrange("p (c j) -> p c j", c=nb))
                                out_dmas.append(od)

    _strip_waits(nc, out_dmas)
    return nc


def _strip_waits(nc, out_dmas):
    """Walrus allows one sync wait per instruction (the final drain takes
    a few). Strip the provably redundant waits:
     - same-engine waits (engines are in-order FIFOs),
     - waits dominated by an earlier same-engine instruction's wait on
       the same semaphore with >= target value (FIFO order covers them),
     - DMASW same-queue WAR waits on DMA copies,
     - the final drain keeps only the last out-DMA's update sems.
    """
    import re
    eng2sem = {"EngineType.DVE": "DVE_",
               "EngineType.Activation": "Activation_",
               "EngineType.Pool": "Pool_",
               "EngineType.PE": "PE_",
               "EngineType.SP": "SP_"}
    # monotonic data-dep sems only: barrier sems reset/decrement, so
    # value-domination logic must never touch them
    _mono = re.compile(r"^(DVE|Activation|Pool|PE|SP)_\d+$"
                       r"|^DMA(HW|SW)\d+_\d+$")

    def _wait_val(w):
        return w.wait_value if w.wait_value is not None else -1

    # both out DMAs ride the same queue (in-order rings), so the last
    # one's completion implies the first's: the drain needs only the
    # last od's sem (walrus allows a single sync wait, even on drains)
    out_sems = set()
    if out_dmas:
        osi = out_dmas[-1].ins.sync_info
        for u in (osi.on_update if osi else []):
            if u.ant_name:
                out_sems.add(u.ant_name)

    bad = []
    for blk in nc.m.functions[0].blocks:
        covered = {}   # (engine, sem) -> max awaited value
        for inst in blk.instructions:
            si = inst.sync_info
            eng = str(inst.engine)
            if si is None or not si.on_wait:
                continue
            iname = type(inst).__name__
            pref = eng2sem.get(eng)
            if iname == "InstDMACopy":
                # DMA data movement is async w.r.t. the issuing engine's
                # pipeline: same-engine waits must be KEPT (the trigger
                # being later in the FIFO does not mean the producer op
                # has retired). Only same-queue WAR DMASW waits go.
                keep = list(si.on_wait)
                nq = [w for w in keep
                      if not (w.ant_name or "").startswith("DMASW")]
                if nq:
                    keep = nq
            else:
                keep = [w for w in si.on_wait
                        if pref is None
                        or not (w.ant_name or "").startswith(pref)]
            if iname == "InstDrain" and len(keep) > 1:
                sel = [w for w in keep if (w.ant_name or "") in out_sems]
                if sel:
                    keep = sel
            # drop waits dominated by an earlier same-engine wait
            # (monotonic named sems only)
            keep = [w for w in keep
                    if not (w.ant_name and _mono.match(w.ant_name))
                    or covered.get((eng, w.ant_name), -1) < _wait_val(w)]
            # same-semaphore waits collapse to the largest target value
            if len(keep) > 1:
                by_sem = {}
                for w in keep:
                    k = w.ant_name
                    if k not in by_sem or _wait_val(w) > _wait_val(by_sem[k]):
                        by_sem[k] = w
                keep = list(by_sem.values())
            for w in keep:
                if w.ant_name and _mono.match(w.ant_name):
                    k = (eng, w.ant_name)
                    if covered.get(k, -1) < _wait_val(w):
                        covered[k] = _wait_val(w)
            if len(keep) != len(si.on_wait):
                si.on_wait = keep
            if len(keep) > 1 and iname not in ("InstDrain",):
                bad.append((inst.name, iname,
                            [(w.ant_name, _wait_val(w)) for w in keep]))
    assert not bad, f"many-wait instructions remain: {bad}"


def _prep_weights(spline_coeff, spline_scaling):
    # C'[m,i,j] = (1/(6h^3)) * sum_g w[m-g] * coeff[i,j,g] * scaling[i,j]
    h = 2.0 / GRID_SIZE
    c = (spline_coeff.astype(np.float64)
         * spline_scaling.astype(np.float64)[:, :, None])  # [i, j, g]
    cp = np.zeros((NM, IN_DIM, OUT_DIM), np.float64)
    for m in range(NM):
        for g in range(max(0, m - 4), m + 1):
            cp[m] += _W_BINOM[m - g] * c[:, :, g]
    cp *= 1.0 / (6.0 * h ** 3)
    return np.ascontiguousarray(
        cp.reshape(NM * IN_DIM, OUT_DIM).astype(np.float32))


def _np_mm_dtype(mm_dtype_name):
    if mm_dtype_name == "float32":
        return np.float32
    if mm_dtype_name == "float16":
        return np.float16
    if mm_dtype_name == "bfloat16":
        import ml_dtypes
        return ml_dtypes.bfloat16
    raise ValueError(mm_dtype_name)


def _run(inputs, trace=False, mm_dtype_name="float16"):
    from concourse.bass_utils import run_bass_kernel_spmd

    key = mm_dtype_name
    if key not in _cached:
        _cached[key] = _build_nc(mm_dtype_name)
    nc = _cached[key]

    x = np.asarray(inputs["x"], np.float32)
    cw = _prep_weights(np.asarray(inputs["spline_coeff"]),
                       np.asarray(inputs["spline_scaling"]))
    cw = np.ascontiguousarray(cw.astype(_np_mm_dtype(mm_dtype_name)))
    in_maps = []
    for c in range(N_CORES):
        xc = np.ascontiguousarray(
            x[c * BC:(c + 1) * BC, :].T.astype(_np_mm_dtype(mm_dtype_name)))
        in_maps.append({"xt": xc, "cw": cw})
    res = run_bass_kernel_spmd(nc, in_maps, list(range(N_CORES)),
                               trace=trace)
    outp = np.concatenate([res.results[c]["out"] for c in range(N_CORES)],
                          axis=0).astype(np.float32)
    return outp, res


def kernel(**inputs):
    outp, _ = _run(inputs, trace=False)
    return outp
